# revision 13
# baseline (speedup 1.0000x reference)
"""Trainium2 Bass kernel for a 6-layer DeBERTa-style encoder (nn_Encoder_12532714570575).

Sharding: data-parallel over batch B=8 -> one batch element per NeuronCore.
Per core: full forward on [S=512, H=768]. No collectives.

Key design (v2):
 - Disentangled-attention position terms via Toeplitz expansion staged in DRAM,
   but BOTH skew reads use the contiguous-per-partition orientation; the
   content->position term is read [s,t]-oriented and transposed on the PE
   directly into the scores PSUM accumulation (transpose-accumulate).
 - Expansion matmuls restricted to the 640-wide used band per 128-row block.
 - Scores assembled in PSUM by PE: cc matmul (start=True), then an
   identity-matmul of the PE-transposed content-position tiles and an
   identity-matmul of the position-content skew read.
 - Residual / DWA stream in fp16; DWA slots in DRAM fp16, one batched DMA per
   slot; mixes as full-width two-accumulator DVE chains.
 - qk bias folded into the PSUM->SBUF copy as a per-partition scalar add.
"""

import sys

sys.path.insert(0, "/opt/trn_rl_repo")

import numpy as np

S, B, HID, NH, HD = 512, 8, 768, 12, 64
L, INT = 6, 2048
K = 63
EPS = 1e-7
SCALE = 1.0 / float(np.sqrt(3 * HD))
NSC = S // 128
NKC = HID // 128
JW = 1024
EW = 640  # staged expansion window per 128-row block

_CACHE = {}


def _build_nc():
    import concourse.bacc as bacc
    import concourse.mybir as mybir
    from concourse import tile
    from concourse.bass import AP

    dt = mybir.dt
    f32, f16 = dt.float32, dt.float16
    AF = mybir.ActivationFunctionType
    ALU = mybir.AluOpType

    nc = bacc.Bacc()

    x_in = nc.dram_tensor("x0", [S, HID], f32, kind="ExternalInput")
    wqk_in = nc.dram_tensor("wqkT", [L, HID, 2 * HID], f16, kind="ExternalInput")
    bqk_in = nc.dram_tensor("bqkT", [L, 128, 12], f32, kind="ExternalInput")
    wvg_in = nc.dram_tensor("wvgT", [L, HID + 1, 2 * HID], f16, kind="ExternalInput")
    wout_in = nc.dram_tensor("woutT", [L, HID + 1, HID], f16, kind="ExternalInput")
    wff1_in = nc.dram_tensor("wff1T", [L, HID, 2 * INT], f16, kind="ExternalInput")
    wff2_in = nc.dram_tensor("wff2T", [L, INT, HID], f16, kind="ExternalInput")
    ke1_in = nc.dram_tensor("ke1r", [L, NH, HD, JW], f16, kind="ExternalInput")
    ke2_in = nc.dram_tensor("ke2", [L, NH, HD, JW], f16, kind="ExternalInput")
    alph_in = nc.dram_tensor("alphrep", [128, 12 * 16], f32, kind="ExternalInput")
    id_in = nc.dram_tensor("id128", [128, 128], f16, kind="ExternalInput")

    out = nc.dram_tensor("out", [L + 1, S, HID], f32, kind="ExternalOutput")

    # per-head staging for the Toeplitz band, double-buffered by layer parity
    c1d = [[nc.dram_tensor(f"c1_{h}_{p}", [S, EW], f16) for p in range(2)]
           for h in range(NH)]
    c2d = [[nc.dram_tensor(f"c2_{h}_{p}", [S, EW], f16) for p in range(2)]
           for h in range(NH)]
    csd = nc.dram_tensor("csd", [NH, S], f16)
    accd = nc.dram_tensor("accd", [2 * L + 1, S, HID], f16)

    def mm(ps, lhsT, rhs, start, stop, **kw):
        nc.tensor.matmul(ps, lhsT, rhs, start=start, stop=stop, **kw)

    from contextlib import ExitStack

    with tile.TileContext(nc) as tc, ExitStack() as stk:
        stk_pools = {}

        def pool(name, bufs, space="SBUF"):
            if name not in stk_pools:
                stk_pools[name] = stk.enter_context(
                    tc.tile_pool(name=name, bufs=bufs, space=space))
            return stk_pools[name]

        cpool = pool("const", 1)
        id_sb = cpool.tile([128, 128], f16, tag="id")
        nc.sync.dma_start(out=id_sb[:], in_=id_in[:])
        alph_sb = cpool.tile([128, 12 * 16], f32, tag="alph")
        nc.sync.dma_start(out=alph_sb[:], in_=alph_in[:])
        ones_row = cpool.tile([1, S], f16, tag="onesr")
        nc.vector.memset(ones_row[:], 1.0)
        ones_col = cpool.tile([128, 1], f16, tag="onesc")
        nc.vector.memset(ones_col[:], 1.0)
        eps_sb = cpool.tile([128, 1], f32, tag="eps")
        nc.vector.memset(eps_sb[:], EPS)
        bqk_sb = cpool.tile([128, 12 * L], f32, tag="bqk")
        for li in range(L):
            nc.sync.dma_start(out=bqk_sb[:, li * 12:(li + 1) * 12], in_=bqk_in[li])

        xp = pool("xp", 2)
        x_sb = xp.tile([128, NSC * HID], f16, tag="x")
        nc.gpsimd.dma_start(
            out=AP(x_sb.tensor, 0, [[NSC * HID, 128], [HID, NSC], [1, HID]]),
            in_=AP(x_in, 0, [[HID, 128], [128 * HID, NSC], [1, HID]]))
        for sc in range(NSC):
            nc.sync.dma_start(out=out[0, sc * 128:(sc + 1) * 128, :],
                              in_=x_in[sc * 128:(sc + 1) * 128, :])
        nc.sync.dma_start(
            out=AP(accd, 0, [[HID, 128], [128 * HID, NSC], [1, HID]]),
            in_=AP(x_sb.tensor, 0, [[NSC * HID, 128], [HID, NSC], [1, HID]]))

        # PSUM pools: mmps(4: projections, FF, scores) + eb(3: transpose
        # batches + expansion chunks) + ctx(1) = 8 banks
        psp = pool("ps", 4, "PSUM")
        peb = pool("peb", 3, "PSUM")
        psctx = pool("psctx", 1, "PSUM")

        lnp = pool("lnp", 1)
        ln4p = pool("ln4p", 2)
        htp = pool("htp", 1)
        stat = pool("stat", 3)

        def layer_norm(src_sb, D, tag):
            """src_sb [128, NSC*D] -> f16 normalized, batched stat tails."""
            ln_sb = lnp.tile([128, NSC * HID], f16, tag="ln", name="ln_" + tag)
            nchk = (D + 511) // 512
            st = stat.tile([128, NSC * 4 * 6], f32, tag="bst", name="bst")
            mvall = stat.tile([128, NSC * 2], f32, tag="mv", name="mv")
            for sc in range(NSC):
                for c in range(nchk):
                    w = min(512, D - c * 512)
                    nc.vector.bn_stats(st[:, (sc * 4 + c) * 6:(sc * 4 + c + 1) * 6],
                                       src_sb[:, sc * D + c * 512: sc * D + c * 512 + w])
                nc.vector.bn_aggr(mvall[:, 2 * sc:2 * sc + 2],
                                  st[:, sc * 24:sc * 24 + nchk * 6])
            sd = stat.tile([128, NSC], f32, tag="sd", name="sd")
            nc.scalar.activation(sd[:], AP(mvall.tensor, 1, [[2 * NSC, 128], [2, NSC]]),
                                 AF.Sqrt, bias=eps_sb[:], scale=1.0)
            rstd = stat.tile([128, NSC], f32, tag="rstd", name="rstd")
            nc.vector.reciprocal(rstd[:], sd[:])
            mr = stat.tile([128, NSC], f32, tag="mr", name="mr")
            nc.vector.scalar_tensor_tensor(
                mr[:], AP(mvall.tensor, 0, [[2 * NSC, 128], [2, NSC]]), -1.0, rstd[:],
                op0=ALU.mult, op1=ALU.mult)
            for sc in range(NSC):
                nc.scalar.activation(ln_sb[:, sc * D:(sc + 1) * D],
                                     src_sb[:, sc * D:(sc + 1) * D],
                                     AF.Identity, bias=mr[:, sc:sc + 1],
                                     scale=rstd[:, sc:sc + 1])
            return ln_sb

        def transpose_768(ln_sb, tag):
            """-> 6 tiles [128, S] (features on partitions), batched copies."""
            hts = [htp.tile([128, S], f16, tag=f"ht{c}", name=f"ht{c}") for c in range(NKC)]
            for c in range(NKC):
                ps = peb.tile([128, S], f16, tag="eb", name="tpb")
                for sc in range(NSC):
                    nc.tensor.matmul(ps[:, sc * 128:(sc + 1) * 128],
                                     ln_sb[:, sc * HID + c * 128: sc * HID + (c + 1) * 128],
                                     id_sb[:], is_transpose=True,
                                     start=(sc == 0), stop=(sc == NSC - 1),
                                     skip_group_check=True)
                nc.vector.tensor_copy(hts[c][:], ps[:])
            return hts

        wbig = pool("wbig", 1)
        wrow = pool("wrow", 1)
        qkvp = pool("qkv", 1)
        attp = pool("attp", 1)
        aop = pool("aop", 2)
        kep = pool("kep", 2)
        stgp = pool("stgp", 3)
        sktp = pool("sktp", 2)
        expp = pool("expp", 4)
        ctxp = pool("ctxp", 4)
        accp = pool("accp", 2)
        ffp = pool("ffp", 1)
        ff2p = pool("ff2p", 1)
        wsp = pool("wsp", 3)

        for li in range(L):
            par = li % 2
            # ===================== attention =====================
            ln1 = layer_norm(x_sb, HID, "ln1")
            hts = transpose_768(ln1, "ht")

            wq = [wbig.tile([128, 2 * HID], f16, tag=f"wq{c}", name=f"wq{c}") for c in range(NKC)]
            for c in range(NKC):
                nc.sync.dma_start(out=wq[c][:], in_=wqk_in[li, c * 128:(c + 1) * 128, :])
            qkT = qkvp.tile([128, 12 * S], f16, tag="qkT")
            for mi in range(12):
                ps = psp.tile([128, S], f32, tag="mmps", name="mmps")
                for c in range(NKC):
                    mm(ps[:], wq[c][:, mi * 128:(mi + 1) * 128], hts[c][:], c == 0, c == NKC - 1)
                nc.vector.tensor_scalar(qkT[:, mi * S:(mi + 1) * S], ps[:],
                                        bqk_sb[:, li * 12 + mi: li * 12 + mi + 1],
                                        None, op0=ALU.add)

            wv = [wbig.tile([128, 2 * HID], f16, tag=f"wq{c}", name=f"wv{c}") for c in range(NKC)]
            for c in range(NKC):
                nc.sync.dma_start(out=wv[c][:], in_=wvg_in[li, c * 128:(c + 1) * 128, :])
            wvb = wrow.tile([1, 2 * HID], f16, tag="wqb", name="wvb")
            nc.sync.dma_start(out=wvb[:], in_=wvg_in[li, HID:HID + 1, :])
            v_sb = qkvp.tile([128, NSC * HID], f16, tag="v")
            g_sb = qkvp.tile([128, NSC * HID], f16, tag="g")
            for sc in range(NSC):
                for ni in range(3):
                    ps = psp.tile([128, S], f32, tag="mmps", name="mmps")
                    for c in range(NKC):
                        mm(ps[:], hts[c][:, sc * 128:(sc + 1) * 128],
                           wv[c][:, ni * 512:(ni + 1) * 512], c == 0, False)
                    mm(ps[:], ones_row[:, sc * 128:(sc + 1) * 128],
                       wvb[:, ni * 512:(ni + 1) * 512], False, True)
                    if ni < 1:
                        nc.vector.tensor_copy(v_sb[:, sc * HID: sc * HID + 512], ps[:])
                    elif ni == 1:
                        nc.vector.tensor_copy(v_sb[:, sc * HID + 512: sc * HID + 768],
                                              ps[:, 0:256])
                        nc.scalar.activation(g_sb[:, sc * HID: sc * HID + 256],
                                             ps[:, 256:512], AF.Gelu)
                    else:
                        nc.scalar.activation(g_sb[:, sc * HID + 256: sc * HID + 768],
                                             ps[:], AF.Gelu)

            # ---- Toeplitz band expansion ----
            for hp in range(NH // 2):
                he = 2 * hp
                ke1 = kep.tile([128, JW], f16, tag="ke1", name="ke1")
                nc.sync.dma_start(
                    out=ke1[:],
                    in_=AP(ke1_in, (li * NH + he) * HD * JW, [[JW, 128], [1, JW]]))
                ke2 = kep.tile([128, JW], f16, tag="ke2", name="ke2")
                nc.sync.dma_start(
                    out=ke2[:],
                    in_=AP(ke2_in, (li * NH + he) * HD * JW, [[JW, 128], [1, JW]]))
                for e in range(2):
                    h = he + e
                    hb = e * 64
                    qh = qkT[hb:hb + 64, hp * S:(hp + 1) * S]
                    kh = qkT[hb:hb + 64, (6 + hp) * S:(6 + hp + 1) * S]
                    for src_i, (src, ke, dst) in enumerate(
                            ((qh, ke1, c1d), (kh, ke2, c2d))):
                        stg = stgp.tile([128, NSC * EW], f16, tag="stg", name="stg")
                        for sc in range(NSC):
                            jlo = 384 - sc * 128
                            psa = peb.tile([128, 512], f32, tag="eb", name="psa")
                            psb = peb.tile([128, 128], f32, tag="eb", name="psb")
                            mm(psa[:], src[:, sc * 128:(sc + 1) * 128],
                               ke[hb:hb + 64, jlo:jlo + 512], True, True,
                               skip_group_check=True)
                            mm(psb[:], src[:, sc * 128:(sc + 1) * 128],
                               ke[hb:hb + 64, jlo + 512:jlo + EW], True, True,
                               skip_group_check=True)
                            if (sc + src_i) % 2 == 0:
                                nc.vector.tensor_copy(stg[:, sc * EW:sc * EW + 512], psa[:])
                                nc.vector.tensor_copy(stg[:, sc * EW + 512:(sc + 1) * EW], psb[:])
                            else:
                                nc.scalar.copy(stg[:, sc * EW:sc * EW + 512], psa[:])
                                nc.scalar.copy(stg[:, sc * EW + 512:(sc + 1) * EW], psb[:])
                        nc.sync.dma_start(
                            out=AP(dst[h][par], 0, [[EW, 128], [128 * EW, NSC], [1, EW]]),
                            in_=AP(stg.tensor, 0, [[NSC * EW, 128], [EW, NSC], [1, EW]]))

            # ---- scores / softmax / ctx per head ----
            ctxg = attp.tile([128, NSC * HID], f16, tag="ctxg")
            for h in range(NH):
                hb = (h % 2) * 64
                hp = h // 2
                qh = qkT[hb:hb + 64, hp * S:(hp + 1) * S]
                kh = qkT[hb:hb + 64, (6 + hp) * S:(6 + hp + 1) * S]
                # skew reads: both contiguous-per-partition
                skt1 = sktp.tile([128, NSC * S], f16, tag="skt1", name="skt1")
                nc.sync.dma_start(
                    out=AP(skt1.tensor, 0, [[NSC * S, 128], [S, NSC], [1, S]]),
                    in_=AP(c1d[h][par], 127, [[EW - 1, 128], [128 * EW, NSC], [1, S]]))
                skt2 = sktp.tile([128, NSC * S], f16, tag="skt2", name="skt2")
                nc.sync.dma_start(
                    out=AP(skt2.tensor, 0, [[NSC * S, 128], [S, NSC], [1, S]]),
                    in_=AP(c2d[h][par], 127, [[EW - 1, 128], [128 * EW, NSC], [1, S]]))
                cps = psctx.tile([65, S], f32, tag="ctxps", name="ctxps")
                for tcb in range(NSC):
                    # transpose the [s,t]-oriented a_cp blocks -> [t,s]
                    trb = peb.tile([128, S], f16, tag="eb", name="trb")
                    for sc in range(NSC):
                        nc.tensor.matmul(
                            trb[:, sc * 128:(sc + 1) * 128],
                            skt1[:, sc * S + tcb * 128: sc * S + (tcb + 1) * 128],
                            id_sb[:], is_transpose=True, start=(sc == 0),
                            stop=(sc == NSC - 1), skip_group_check=True)
                    acp = expp.tile([128, S], f16, tag="acp", name="acp")
                    nc.vector.tensor_copy(acp[:], trb[:])
                    ps = psp.tile([128, S], f32, tag="mmps", name="scps")
                    mm(ps[:], kh[:, tcb * 128:(tcb + 1) * 128], qh[:], True, False,
                       skip_group_check=True)
                    mm(ps[:], id_sb[:], acp[:], False, False, skip_group_check=True)
                    mm(ps[:], id_sb[:], skt2[:, tcb * S:(tcb + 1) * S], False, True,
                       skip_group_check=True)
                    ex = expp.tile([128, S], f16, tag="exp", name="exp")
                    nc.scalar.activation(ex[:], ps[:], AF.Exp, scale=SCALE)
                    vslice = v_sb[:, tcb * HID + h * 64: tcb * HID + h * 64 + 64]
                    mm(cps[0:64, :], vslice, ex[:], tcb == 0, tcb == NSC - 1,
                       skip_group_check=True)
                    mm(cps[64:65, :], ones_col[:], ex[:], tcb == 0, tcb == NSC - 1,
                       skip_group_check=True)
                ctxT_h = ctxp.tile([64, S], f16, tag="ctxT", name=f"ctxT{h}")
                nc.vector.tensor_copy(ctxT_h[:], cps[0:64, :])
                csh = stat.tile([65, S], f16, tag="csh", name="csh", bufs=2)
                nc.scalar.copy(csh[64:65, :], cps[64:65, :])
                nc.sync.dma_start(out=csd[h, :], in_=csh[64:65, :])
                # per-head gating: transpose ctx back to [s, d], scale by
                # 1/colsum and by gelu gate
                rch = stat.tile([128, NSC], f16, tag="cst", name="rch", bufs=2)
                nc.sync.dma_start(out=rch[:], in_=AP(csd, h * S, [[1, 128], [128, NSC]]))
                rcp = stat.tile([128, NSC], f32, tag="rcp", name="rcp", bufs=2)
                nc.vector.reciprocal(rcp[:], rch[:])
                pst = peb.tile([128, S], f16, tag="eb", name="pst")
                for sc in range(NSC):
                    nc.tensor.matmul(pst[:, sc * 128:sc * 128 + 64],
                                     ctxT_h[:, sc * 128:(sc + 1) * 128],
                                     id_sb[0:64, 0:64], is_transpose=True,
                                     start=(sc == 0), stop=(sc == NSC - 1),
                                     skip_group_check=True)
                for sc in range(NSC):
                    nc.vector.scalar_tensor_tensor(
                        ctxg[:, sc * HID + h * 64: sc * HID + (h + 1) * 64],
                        pst[:, sc * 128:sc * 128 + 64],
                        rcp[:, sc:sc + 1],
                        g_sb[:, sc * HID + h * 64: sc * HID + (h + 1) * 64],
                        op0=ALU.mult, op1=ALU.mult)

            ln2 = layer_norm(ctxg, HID, "ln2")
            l2t = transpose_768(ln2, "ht")
            wo = [wbig.tile([128, HID], f16, tag=f"wq{c}", name=f"wo{c}") for c in range(NKC)]
            for c in range(NKC):
                nc.sync.dma_start(out=wo[c][:], in_=wout_in[li, c * 128:(c + 1) * 128, :])
            wob = wrow.tile([1, HID], f16, tag="wqb", name="wob")
            nc.sync.dma_start(out=wob[:], in_=wout_in[li, HID:HID + 1, :])
            att_out = aop.tile([128, NSC * HID], f16, tag="ao", name="attout")
            for sc in range(NSC):
                for ni, nw in ((0, 512), (1, 256)):
                    ps = psp.tile([128, 512], f32, tag="mmps", name="mmps")
                    for c in range(NKC):
                        mm(ps[:, 0:nw], l2t[c][:, sc * 128:(sc + 1) * 128],
                           wo[c][:, ni * 512: ni * 512 + nw], c == 0, False)
                    mm(ps[:, 0:nw], ones_row[:, sc * 128:(sc + 1) * 128],
                       wob[:, ni * 512: ni * 512 + nw], False, True)
                    nc.vector.scalar_tensor_tensor(
                        att_out[:, sc * HID + ni * 512: sc * HID + ni * 512 + nw],
                        ps[:, 0:nw], 1.0, x_sb[:, sc * HID + ni * 512: sc * HID + ni * 512 + nw],
                        op0=ALU.mult, op1=ALU.add)
            nc.sync.dma_start(
                out=AP(accd, (2 * li + 1) * S * HID, [[HID, 128], [128 * HID, NSC], [1, HID]]),
                in_=AP(att_out.tensor, 0, [[NSC * HID, 128], [HID, NSC], [1, HID]]))

            def dwa_mix(row, nslots, newest_sb):
                """x_new = sum_j alpha[row, j] * acc[j]; 3-way split accumulation."""
                xn = xp.tile([128, NSC * HID], f16, tag="x", name="xn")
                old = nslots - 1
                slots = []
                for j in range(old):
                    t = accp.tile([128, NSC * HID], f16, tag="accl", name="accl")
                    nc.sync.dma_start(
                        out=AP(t.tensor, 0, [[NSC * HID, 128], [HID, NSC], [1, HID]]),
                        in_=AP(accd, j * S * HID, [[HID, 128], [128 * HID, NSC], [1, HID]]))
                    slots.append(t)

                def acc_into(eng, dst, items, init_with_newest):
                    first = True
                    if init_with_newest:
                        eng.tensor_scalar(dst, newest_sb[:],
                                          alph_sb[:, row * 16 + nslots - 1: row * 16 + nslots],
                                          None, op0=ALU.mult)
                        first = False
                    for j in items:
                        a_ap = alph_sb[:, row * 16 + j: row * 16 + j + 1]
                        if first:
                            eng.tensor_scalar(dst, slots[j][:], a_ap, None, op0=ALU.mult)
                            first = False
                        else:
                            eng.scalar_tensor_tensor(dst, slots[j][:], a_ap, dst,
                                                     op0=ALU.mult, op1=ALU.add)

                if old <= 3:
                    acc_into(nc.vector, xn[:], list(range(old)), True)
                else:
                    part = accp.tile([128, NSC * HID], f16, tag="mixp", name="mixp", bufs=1)
                    dv = [j for j in range(old) if j % 3 != 2]
                    gp = [j for j in range(old) if j % 3 == 2]
                    acc_into(nc.vector, xn[:], dv, True)
                    acc_into(nc.vector, part[:], gp, False)
                    nc.vector.tensor_tensor(xn[:], xn[:], part[:], ALU.add)
                return xn

            x_sb = dwa_mix(2 * li, 2 * li + 2, att_out)

            # ===================== GeGLU FFN =====================
            ln3 = layer_norm(x_sb, HID, "ln3")
            l3t = transpose_768(ln3, "ht")
            w_sb = ffp.tile([128, NSC * INT], f16, tag="wact")
            for nchunk in range(8):
                wt = wsp.tile([128, 512], f16, tag="wff1", name="wt")
                nc.sync.dma_start(out=wt[:], in_=wff1_in[li, 0:128, nchunk * 512:(nchunk + 1) * 512])
                pss = [psp.tile([128, 512], f32, tag="mmps", name=f"ps{sc}", bufs=4)
                       for sc in range(NSC)]
                for c in range(NKC):
                    if c > 0:
                        wt = wsp.tile([128, 512], f16, tag="wff1", name="wt")
                        nc.sync.dma_start(out=wt[:], in_=wff1_in[li, c * 128:(c + 1) * 128,
                                                                 nchunk * 512:(nchunk + 1) * 512])
                    for sc in range(NSC):
                        mm(pss[sc][:], l3t[c][:, sc * 128:(sc + 1) * 128], wt[:],
                           c == 0, c == NKC - 1)
                for sc in range(NSC):
                    if nchunk < 4:
                        nc.vector.tensor_copy(w_sb[:, sc * INT + nchunk * 512: sc * INT + (nchunk + 1) * 512],
                                              pss[sc][:])
                    else:
                        g2 = ffp.tile([128, 512], f16, tag="g2g", bufs=2, name="g2")
                        nc.scalar.activation(g2[:], pss[sc][:], AF.Gelu_apprx_tanh)
                        col = sc * INT + (nchunk - 4) * 512
                        nc.vector.tensor_tensor(w_sb[:, col:col + 512],
                                                w_sb[:, col:col + 512], g2[:], ALU.mult)

            # LN4 + transpose, per sc to bound SBUF
            l4t = [ff2p.tile([128, INT], f16, tag=f"l4t{sc}", name=f"l4t{sc}") for sc in range(NSC)]
            nchk4 = INT // 512
            st4 = stat.tile([128, NSC * 4 * 6], f32, tag="bst", name="bst4")
            mv4 = stat.tile([128, NSC * 2], f32, tag="mv", name="mv4")
            for sc in range(NSC):
                for c in range(nchk4):
                    nc.vector.bn_stats(st4[:, (sc * 4 + c) * 6:(sc * 4 + c + 1) * 6],
                                       w_sb[:, sc * INT + c * 512: sc * INT + (c + 1) * 512])
                nc.vector.bn_aggr(mv4[:, 2 * sc:2 * sc + 2], st4[:, sc * 24:sc * 24 + nchk4 * 6])
            sd4 = stat.tile([128, NSC], f32, tag="sd", name="sd4")
            nc.scalar.activation(sd4[:], AP(mv4.tensor, 1, [[2 * NSC, 128], [2, NSC]]),
                                 AF.Sqrt, bias=eps_sb[:], scale=1.0)
            rstd4 = stat.tile([128, NSC], f32, tag="rstd", name="rstd4")
            nc.vector.reciprocal(rstd4[:], sd4[:])
            mr4 = stat.tile([128, NSC], f32, tag="mr", name="mr4")
            nc.vector.scalar_tensor_tensor(
                mr4[:], AP(mv4.tensor, 0, [[2 * NSC, 128], [2, NSC]]), -1.0, rstd4[:],
                op0=ALU.mult, op1=ALU.mult)
            for sc in range(NSC):
                ln4sc = ln4p.tile([128, INT], f16, tag="ln4", name="ln4sc")
                nc.scalar.activation(ln4sc[:], w_sb[:, sc * INT:(sc + 1) * INT],
                                     AF.Identity, bias=mr4[:, sc:sc + 1],
                                     scale=rstd4[:, sc:sc + 1])
                for kg in range(4):
                    ps = peb.tile([128, 512], f16, tag="eb", name="l4b")
                    for k2 in range(4):
                        kc = kg * 4 + k2
                        nc.tensor.matmul(ps[:, k2 * 128:(k2 + 1) * 128],
                                         ln4sc[:, kc * 128:(kc + 1) * 128], id_sb[:],
                                         is_transpose=True, start=(k2 == 0), stop=(k2 == 3),
                                         skip_group_check=True)
                    nc.vector.tensor_copy(l4t[sc][:, kg * 512:(kg + 1) * 512], ps[:])

            ffn_out = aop.tile([128, NSC * HID], f16, tag="ao", name="ffnout")
            for ni, nw in ((0, 512), (1, 256)):
                pss = [psp.tile([128, 512], f32, tag="mmps", name=f"ps{sc}", bufs=4)
                       for sc in range(NSC)]
                for kc in range(16):
                    wt = wsp.tile([128, 512], f16, tag="wff1", name="wt")
                    nc.sync.dma_start(out=wt[:, 0:nw], in_=wff2_in[li, kc * 128:(kc + 1) * 128,
                                                                   ni * 512: ni * 512 + nw])
                    for sc in range(NSC):
                        mm(pss[sc][:, 0:nw], l4t[sc][:, kc * 128:(kc + 1) * 128],
                           wt[:, 0:nw], kc == 0, kc == 15)
                for sc in range(NSC):
                    nc.vector.scalar_tensor_tensor(
                        ffn_out[:, sc * HID + ni * 512: sc * HID + ni * 512 + nw],
                        pss[sc][:, 0:nw], 1.0,
                        x_sb[:, sc * HID + ni * 512: sc * HID + ni * 512 + nw],
                        op0=ALU.mult, op1=ALU.add)
            nc.sync.dma_start(
                out=AP(accd, (2 * li + 2) * S * HID, [[HID, 128], [128 * HID, NSC], [1, HID]]),
                in_=AP(ffn_out.tensor, 0, [[NSC * HID, 128], [HID, NSC], [1, HID]]))

            x_sb = dwa_mix(2 * li + 1, 2 * li + 3, ffn_out)
            nc.gpsimd.dma_start(
                out=AP(out, (li + 1) * S * HID, [[HID, 128], [128 * HID, NSC], [1, HID]]),
                in_=AP(x_sb.tensor, 0, [[NSC * HID, 128], [HID, NSC], [1, HID]]))

    nc.finalize()
    return nc


def _host_prep(inputs):
    x = np.asarray(inputs["x"], np.float32)
    rel = np.asarray(inputs["relative_embedding"], np.float64)
    pos = np.asarray(inputs["position_indices"])
    Wqk = np.asarray(inputs["Wqk"], np.float64)
    bqk = np.asarray(inputs["bqk"], np.float64)
    Wvg = np.asarray(inputs["Wvg"], np.float64)
    bvg = np.asarray(inputs["bvg"], np.float64)
    Wout = np.asarray(inputs["Wout"], np.float64)
    bout = np.asarray(inputs["bout"], np.float64)
    Wff1 = np.asarray(inputs["Wff1"], np.float32)
    Wff2 = np.asarray(inputs["Wff2"], np.float32)
    alphas = np.asarray(inputs["alphas"], np.float32)

    d = np.arange(-511, 512)
    F = np.where(d >= 0, pos[np.maximum(d, 0), 0], pos[0, np.maximum(-d, 0)]).astype(np.int64)

    wqkT = np.ascontiguousarray(Wqk.transpose(0, 2, 1)).astype(np.float16)
    bqkT = np.zeros((L, 128, 12), np.float32)
    for mi in range(12):
        bqkT[:, :, mi] = bqk[:, mi * 128:(mi + 1) * 128]
    wvgT = np.concatenate([Wvg.transpose(0, 2, 1), bvg[:, None, :]], axis=1).astype(np.float16)
    woutT = np.concatenate([Wout.transpose(0, 2, 1), bout[:, None, :]], axis=1).astype(np.float16)
    wff1T = np.ascontiguousarray(Wff1.transpose(0, 2, 1)).astype(np.float16)
    wff2T = np.ascontiguousarray(Wff2.transpose(0, 2, 1)).astype(np.float16)

    ke1 = np.zeros((L, NH, HD, JW), np.float16)
    ke2 = np.zeros((L, NH, HD, JW), np.float16)
    for li in range(L):
        proj = rel @ Wqk[li].T + bqk[li]
        qpos = proj[:, :HID].reshape(K, NH, HD)
        kpos = proj[:, HID:].reshape(K, NH, HD)
        ke1[li, :, :, 0:1023] = kpos[F[::-1]].transpose(1, 2, 0).astype(np.float16)
        ke2[li, :, :, 0:1023] = qpos[F].transpose(1, 2, 0).astype(np.float16)

    alph = np.zeros((128, 12 * 16), np.float32)
    for n in range(12):
        alph[:, n * 16:n * 16 + 13] = alphas[n][None, :]

    base = {
        "wqkT": wqkT, "bqkT": bqkT, "wvgT": wvgT, "woutT": woutT,
        "wff1T": wff1T, "wff2T": wff2T,
        "ke1r": ke1, "ke2": ke2, "alphrep": alph,
        "id128": np.eye(128, dtype=np.float16),
    }
    in_maps = []
    for b in range(B):
        m = dict(base)
        m["x0"] = np.ascontiguousarray(x[:, b, :])
        in_maps.append(m)
    return in_maps


def get_compiled():
    if "nc" not in _CACHE:
        _CACHE["nc"] = _build_nc()
    return _CACHE["nc"]


def kernel(**inputs) -> np.ndarray:
    from concourse.bass_utils import run_bass_kernel_spmd

    nc = get_compiled()
    in_maps = _host_prep(inputs)
    res = run_bass_kernel_spmd(nc, in_maps, list(range(B)))
    outs = [res.results[b]["out"] for b in range(B)]
    return np.stack(outs, axis=2).astype(np.float32)
